# revision 10
# baseline (speedup 1.0000x reference)
"""CapsNet forward as a fused Bass/Tile kernel on 8 Trainium2 NeuronCores.

Math (same collapse as before, validated vs jax reference):
  routing logits never update -> uniform c -> capsule stage = mean over the
  1152 capsules; squash scalars; tiny logits matmul; softmax.

v2 structure (per core, 64 samples, 4 chunks of 16):
  - im2col is done HOST-side: xim[k] = [81, 19, 20, 16] bf16 (x col 19 = 0),
    one contiguous SWDGE DMA per chunk (no gather, no memsets).
  - conv1: per y-row matmul K=81 N=320 (bf16), evicted with relu+bias to
    C1t[T] in PHASE-MAJOR fp8e4 layout [128=(dd,ci), phi, y', xm, xq, b]
    (y = 4*y' + phi, x = 4*xq + xm).  Phase-major makes each (T,dd) shuffle
    source a single contiguous 6400B run per partition.
  - shuffle: 8 DMAs/chunk (one per T,dd): C1t[T][32dd:32dd+32, :] ->
    C1ph[:, d] with dst partitions ordered (ci,phi) = 4*ci+phi; contiguous
    1600B runs both sides.
  - prim caps conv: fp8e4 DoubleRow; the 33 K-tiles pair into 16 DR pairs
    (11 seg0+seg1 pairs, 5 seg2 xh-pairs) + 1 single.  Weights prescaled
    x64 host-side; oy-inner loop reuses each loaded weight pair for 3 MMs.
    Evict relu(psum/64 + b) to U bf16.
  - dig projection / squash / logits / softmax: as before.
"""
import sys

sys.path.insert(0, "/opt/trn_rl_repo")

import numpy as np
import ml_dtypes

N_CORES = 8
B = 512
BC = B // N_CORES        # 64 samples per core
BCH = 16                 # batch chunk
NCH = BC // BCH          # 4 chunks

W2SCALE = 64.0

# DR pair table: 16 pairs + 1 single over the 33 prim K-tiles (xh, seg).
# pairs i<11: ((i,0),(i,1)) stride = y'+1; pairs 11..15: ((2j,2),(2j+1,2))
# stride = xm+1; single: (10,2).
N_PAIRS = 16


def _pair_members(i):
    if i < 11:
        return (i, 0), (i, 1)
    j = i - 11
    return (2 * j, 2), (2 * j + 1, 2)


# ---------------------------------------------------------------- host tables
def _build_tables(conv1_w, conv1_b, prim_w, prim_b, dig_W, dig_Wb, out_w, out_b):
    bf = ml_dtypes.bfloat16
    f8 = ml_dtypes.float8_e4m3
    w1 = conv1_w[:, 0].reshape(256, 81)
    w2 = prim_w[:, :, 0]                       # [co, ci, p, q]

    W1r = np.zeros((81, 256), np.float32)
    Cbias = np.zeros((128, 2), np.float32)
    for d in range(8):
        for ci in range(32):
            c = ci * 8 + d
            T, mu = d // 4, 32 * (d % 4) + ci
            W1r[:, T * 128 + mu] = w1[c]
            Cbias[mu, T] = conv1_b[c]

    # W2stk[row=(4*ci+phi), pair, member, col=(32*s+co)] fp8, x64
    W2stk = np.zeros((128, 17, 2, 128), np.float32)

    def fill_tile(pr, m, xh, seg):
        nphi = 4 if seg < 2 else 3
        for phi in range(nphi):
            for s in range(4):
                sy, sx = s // 2, s % 2
                p = 4 * seg + phi - 2 * sy
                q = xh - 2 * sx
                if 0 <= p <= 8 and 0 <= q <= 8:
                    for ci in range(32):
                        W2stk[4 * ci + phi, pr, m, 32 * s:32 * s + 32] = \
                            w2[:, ci, p, q] * W2SCALE

    for i in range(N_PAIRS):
        (xa, sa), (xb, sb) = _pair_members(i)
        fill_tile(i, 0, xa, sa)
        fill_tile(i, 1, xb, sb)
    fill_tile(16, 0, 10, 2)

    Pbias = np.zeros((128, 1), np.float32)
    for s in range(4):
        Pbias[32 * s:32 * s + 32, 0] = prim_b

    Wdig = np.zeros((72, 128, 16), np.float32)
    t = 0
    for oy in range(3):
        for d in range(8):
            for ox in range(3):
                for s in range(4):
                    sy, sx = s // 2, s % 2
                    ip, jp = 2 * oy + sy, 2 * ox + sx
                    for co in range(32):
                        n = co * 36 + jp * 6 + ip
                        Wdig[t, 32 * s + co] = dig_W[n, d] / 1152.0
                t += 1

    return dict(
        W1r=W1r.astype(bf),
        Cbias=Cbias,
        W2stk=W2stk.astype(f8),
        Pbias=Pbias,
        Wdig=np.ascontiguousarray(Wdig.transpose(1, 0, 2)).astype(bf),  # [128,72,16]
        Dbias=(dig_Wb.sum(0) / 1152.0).reshape(16, 1).astype(np.float32),
        W2sT=np.ascontiguousarray(out_w[..., 0].sum(1).T).astype(np.float32),
        ob=np.tile(out_b[None, :], (BC, 1)).astype(np.float32),
    )


def _build_xim(x):
    """Host im2col: per core, [NCH, 81, 19, 20, BCH] bf16, x col 19 zeroed."""
    from numpy.lib.stride_tricks import sliding_window_view
    xi = x[:, 0]                                        # [B, 28, 28] f32
    out = []
    for c in range(N_CORES):
        xc = xi[c * BC:(c + 1) * BC]                    # [64, 28, 28]
        w = sliding_window_view(xc, (9, 9), axis=(1, 2))[:, :19, :19]  # [64,19,19,9,9]
        im = np.zeros((81, 19, 20, NCH, BCH), np.float32)
        im[:, :, :19] = (
            w.transpose(3, 4, 1, 2, 0).reshape(81, 19, 19, NCH, BCH)
        )
        out.append(np.ascontiguousarray(
            im.transpose(3, 0, 1, 2, 4)).astype(ml_dtypes.bfloat16))
    return out                                           # list of [4,81,19,20,16]


# ---------------------------------------------------------------- bass kernel
def _build_nc():
    import concourse.bacc as bacc
    import concourse.bass as bass
    import concourse.mybir as mybir
    import concourse.tile as tile
    from concourse.masks import make_identity

    bf = mybir.dt.bfloat16
    f8 = mybir.dt.float8e4
    f32 = mybir.dt.float32
    AF = mybir.ActivationFunctionType
    AX = mybir.AxisListType
    DR = mybir.MatmulPerfMode.DoubleRow

    nc = bacc.Bacc(None, target_bir_lowering=False)

    xim_in = nc.dram_tensor("xim", [NCH, 81, 19, 20, BCH], bf, kind="ExternalInput")
    W1r_d = nc.dram_tensor("W1r", [81, 256], bf, kind="ExternalInput")
    W2stk_d = nc.dram_tensor("W2stk", [128, 17, 2, 128], f8, kind="ExternalInput")
    Wdig_d = nc.dram_tensor("Wdig", [128, 72, 16], bf, kind="ExternalInput")
    Cbias_d = nc.dram_tensor("Cbias", [128, 2], f32, kind="ExternalInput")
    Pbias_d = nc.dram_tensor("Pbias", [128, 1], f32, kind="ExternalInput")
    Dbias_d = nc.dram_tensor("Dbias", [16, 1], f32, kind="ExternalInput")
    W2sT_d = nc.dram_tensor("W2sT", [16, 10], f32, kind="ExternalInput")
    ob_d = nc.dram_tensor("ob", [BC, 10], f32, kind="ExternalInput")
    out_d = nc.dram_tensor("out", [BC, 10], f32, kind="ExternalOutput")

    with tile.TileContext(nc) as tc:
        with (
            tc.tile_pool(name="consts", bufs=1) as consts,
            tc.tile_pool(name="work", bufs=2) as work,
            tc.tile_pool(name="usb", bufs=1) as usbp,
            tc.tile_pool(name="fin", bufs=1) as fin,
            tc.tile_pool(name="ps1", bufs=3, space="PSUM") as ps1p,
            tc.tile_pool(name="pspr", bufs=3, space="PSUM") as psprp,
            tc.tile_pool(name="psm", bufs=1, space="PSUM") as psmp,
            tc.tile_pool(name="pssm", bufs=1, space="PSUM") as pssmp,
            nc.allow_non_contiguous_dma("phase shuffle partition regroup"),
        ):
            # ---- constants: small ones first so conv1 can start early;
            # the big prim/dig tables load behind them on the sync ring.
            W1sb = consts.tile([81, 256], bf)
            nc.sync.dma_start(out=W1sb, in_=W1r_d[:, :])
            Cb = consts.tile([128, 2], f32)
            nc.scalar.dma_start(out=Cb, in_=Cbias_d[:, :])
            Pb = consts.tile([128, 1], f32)
            nc.scalar.dma_start(out=Pb, in_=Pbias_d[:, :])
            Db = consts.tile([16, 1], f32)
            nc.scalar.dma_start(out=Db, in_=Dbias_d[:, :])
            W2s = consts.tile([16, 10], f32)
            nc.scalar.dma_start(out=W2s, in_=W2sT_d[:, :])
            obt = consts.tile([BC, 10], f32)
            nc.scalar.dma_start(out=obt, in_=ob_d[:, :])
            W2sb = consts.tile([128, 17, 2, 128], f8)
            nc.sync.dma_start(out=W2sb, in_=W2stk_d[:, :, :, :])
            Wdsb = consts.tile([128, 72, 16], bf)
            nc.sync.dma_start(out=Wdsb, in_=Wdig_d[:, :, :])
            idf = consts.tile([16, 16], f32)
            make_identity(nc, idf)

            U_sb = usbp.tile([128, 3, 8, 3, BC], bf)

            # ---------------- prim caps helpers (fp8 DoubleRow) ----------
            def pair_rhs(C1ph, i, oy):
                if i < 11:
                    xm0, xq0 = i % 4, i // 4
                    s = C1ph[:, :, oy:oy + 2, xm0, xq0:xq0 + 3, :]
                else:
                    xh = 2 * (i - 11)
                    xm0, xq0 = xh % 4, xh // 4
                    s = C1ph[:, :, oy + 2, xm0:xm0 + 2, xq0:xq0 + 3, :]
                return s.transpose([0, 2, 1, 3, 4])

            def emit_prim_mm(C1ph, i, oy, pspr):
                if i < N_PAIRS:
                    nc.tensor.matmul(
                        pspr[:, :, :, :],
                        W2sb[:, i, :, :], pair_rhs(C1ph, i, oy),
                        start=(i == 0), stop=False, perf_mode=DR)
                else:
                    nc.tensor.matmul(
                        pspr[:, :, :, :],
                        W2sb[:, 16, 0, :],
                        C1ph[:, :, oy + 2, 2, 2:5, :],
                        start=False, stop=True)

            def evict_U(k, oy, pspr):
                nc.scalar.activation(
                    out=U_sb[:, oy, :, :, k * BCH:(k + 1) * BCH],
                    in_=pspr, func=AF.Relu, bias=Pb[:, 0:1], scale=1.0 / W2SCALE)

            def prim_items(C1ph, psprs, k):
                # oy-outer: the first 17 MMs (oy=0) read only y' 0:2 of C1ph,
                # i.e. depend on the low-half shuffles only -> a ~17-MM runway
                # that hides the high-half shuffle latency at chunk start.
                for oy in range(3):
                    for i in range(17):
                        yield ('mm', C1ph, i, oy, psprs[oy])
                    yield ('ev', k, oy, psprs[oy])

            def run_item(it):
                if it[0] == 'mm':
                    _, C1ph, i, oy, pspr = it
                    emit_prim_mm(C1ph, i, oy, pspr)
                else:
                    _, k, oy, pspr = it
                    evict_U(k, oy, pspr)

            # ---------------- conv1 + phase-major evict ------------------
            def emit_conv1_T(im1, T, C1t_T, C1ph, inter):
                for y in range(19):
                    ps = ps1p.tile([128, 20, BCH], f32, tag="ps1", name="ps1")
                    nc.tensor.matmul(ps, W1sb[:, 128 * T:128 * (T + 1)],
                                     im1[:, y, :, :])
                    phi, yp = y % 4, y // 4
                    dst = C1t_T[:, phi, yp, :, :, :]
                    src = ps.rearrange("p (xq xm) b -> p xm xq b", xm=4)
                    if y % 2 == 0:
                        nc.scalar.activation(out=dst, in_=src, func=AF.Relu,
                                             bias=Cb[:, T:T + 1], scale=1.0)
                    else:
                        nc.vector.tensor_scalar(
                            out=dst, in0=src, scalar1=Cb[:, T:T + 1],
                            scalar2=0.0, op0=mybir.AluOpType.add,
                            op1=mybir.AluOpType.max)
                    if y == 11:
                        # rows y<=11 = y' 0:3 of every phi are evicted: the
                        # low-half shuffle can start while conv1 finishes
                        emit_shuffle_T(T, C1t_T, C1ph, 0, 3)
                    inter(y)

            def emit_shuffle_T(T, C1t_T, C1ph, lo, hi):
                # y'-slice [lo:hi] of the phase shuffle: the low slice only
                # needs conv1 rows y <= 4*(lo... issued mid-conv1 so most of
                # the shuffle overlaps compute.
                for dd in range(4):
                    d = 4 * T + dd
                    eng = nc.sync if dd % 2 == 0 else nc.scalar
                    eng.dma_start(
                        out=C1ph[:, d, lo:hi, :, :, :],
                        in_=C1t_T[32 * dd:32 * dd + 32, :, lo:hi, :, :, :])

            def emit_im1(k):
                im1 = work.tile([81, 19, 20, BCH], bf, tag="im1", name="im1",
                                bufs=2)
                nc.sync.dma_start(out=im1[0:41, :, :, :],
                                  in_=xim_in[k, 0:41, :, :, :])
                nc.scalar.dma_start(out=im1[41:81, :, :, :],
                                    in_=xim_in[k, 41:81, :, :, :])
                return im1

            # ---------------- chunk loop --------------------------------
            im1s = {0: emit_im1(0)}
            pend = None      # prim work-item iterator for chunk k-1
            for k in range(NCH):
                if k + 1 < NCH:
                    im1s[k + 1] = emit_im1(k + 1)
                im1 = im1s.pop(k)
                C1t = [work.tile([128, 4, 5, 4, 5, BCH], f8, tag=f"c1t{T}",
                                 name=f"c1t{T}") for T in range(2)]
                if k < 2:
                    # zero the phi3/y'4 pad slot once per pool buffer so the
                    # shuffle copies finite data into C1ph's unused slots
                    # (read with zero weights by the seg2 K-tiles).
                    for T in range(2):
                        nc.gpsimd.memset(C1t[T][:, 3, 4, :, :, :], 0.0)
                C1ph = work.tile([128, 8, 5, 4, 5, BCH], f8, tag="c1ph",
                                 name="c1ph", bufs=3)

                def inter(y):
                    if pend is not None and y % 2 == 1:
                        for _ in range(3):
                            it = next(pend, None)
                            if it is not None:
                                run_item(it)

                emit_conv1_T(im1, 0, C1t[0], C1ph, inter)
                emit_shuffle_T(0, C1t[0], C1ph, 3, 5)
                emit_conv1_T(im1, 1, C1t[1], C1ph, inter)
                emit_shuffle_T(1, C1t[1], C1ph, 3, 5)
                if pend is not None:
                    for it in pend:
                        run_item(it)

                psprs = [psprp.tile([128, 8, 3, BCH], f32, tag="pspr",
                                    name="pspr") for _ in range(3)]
                pend = prim_items(C1ph, psprs, k)

            # ---------------- tail: chunk 3 prim (oy-outer) + dig --------
            psm = psmp.tile([16, BC], f32, tag="psm")
            tdig = [0]

            def dig_gen(oy):
                # yields after emitting each dig matmul so it can interleave
                # with the next oy's prim matmuls
                for d in range(8):
                    for ox in range(3):
                        t = tdig[0]
                        nc.tensor.matmul(psm, Wdsb[:, t, :],
                                         U_sb[:, oy, d, ox, :],
                                         start=(t == 0), stop=(t == 71))
                        tdig[0] += 1
                        yield

            dig_it = None
            for it in pend:
                run_item(it)
                if it[0] == 'mm':
                    if dig_it is not None:
                        next(dig_it, None)
                else:
                    if dig_it is not None:
                        for _ in dig_it:
                            pass
                    dig_it = dig_gen(it[2])
            for _ in dig_it:
                pass

            # ---------------- squash / logits / softmax ------------------
            m_sb = fin.tile([16, BC], f32)
            nc.vector.tensor_scalar_add(out=m_sb, in0=psm, scalar1=Db[:, 0:1])

            psT = pssmp.tile([BC, 16], f32, tag="pssm")
            nc.tensor.transpose(psT, m_sb, idf)
            mT = fin.tile([BC, 16], f32)
            nc.vector.tensor_copy(out=mT, in_=psT)
            sq = fin.tile([BC, 16], f32)
            nc.vector.tensor_mul(sq, mT, mT)
            l2 = fin.tile([BC, 1], f32)
            nc.vector.reduce_sum(l2, sq, axis=AX.X)
            nc.scalar.activation(out=l2, in_=l2, func=AF.Sqrt)
            l1 = fin.tile([BC, 1], f32)
            nc.vector.tensor_reduce(l1, mT, axis=AX.X, op=mybir.AluOpType.add,
                                    apply_absolute_value=True)
            den = fin.tile([BC, 1], f32)
            nc.vector.tensor_scalar_add(out=den, in0=l2, scalar1=1.0)
            nc.vector.tensor_mul(den, den, l1)
            rden = fin.tile([BC, 1], f32)
            nc.vector.reciprocal(rden, den)
            scl = fin.tile([BC, 1], f32)
            nc.vector.tensor_mul(scl, l2, rden)

            pslg = pssmp.tile([BC, 10], f32, tag="pssm")
            nc.tensor.matmul(pslg, m_sb, W2s)
            lg = fin.tile([BC, 10], f32)
            nc.vector.tensor_scalar_mul(out=lg, in0=pslg, scalar1=scl[:, 0:1])
            nc.vector.tensor_add(lg, lg, obt)
            mx = fin.tile([BC, 1], f32)
            nc.vector.reduce_max(mx, lg, axis=AX.X)
            nc.vector.tensor_scalar_sub(out=lg, in0=lg, scalar1=mx[:, 0:1])
            ex = fin.tile([BC, 10], f32)
            nc.scalar.activation(out=ex, in_=lg, func=AF.Exp)
            sm = fin.tile([BC, 1], f32)
            nc.vector.reduce_sum(sm, ex, axis=AX.X)
            rsm = fin.tile([BC, 1], f32)
            nc.vector.reciprocal(rsm, sm)
            outt = fin.tile([BC, 10], f32)
            nc.vector.tensor_scalar_mul(out=outt, in0=ex, scalar1=rsm[:, 0:1])
            nc.sync.dma_start(out=out_d[:, :], in_=outt)

    nc.finalize()
    return nc


_CACHE = {}


def kernel(**inputs):
    from concourse.bass_utils import run_bass_kernel_spmd

    np_in = {k: np.asarray(v) for k, v in inputs.items()}
    tabs = _build_tables(
        np_in["conv1_w"], np_in["conv1_b"], np_in["prim_w"], np_in["prim_b"],
        np_in["dig_W"], np_in["dig_Wb"], np_in["out_w"], np_in["out_b"],
    )
    xims = _build_xim(np_in["x"].astype(np.float32))

    if "nc" not in _CACHE:
        _CACHE["nc"] = _build_nc()
    nc = _CACHE["nc"]

    shared = {
        "W1r": tabs["W1r"], "W2stk": tabs["W2stk"], "Wdig": tabs["Wdig"],
        "Cbias": tabs["Cbias"], "Pbias": tabs["Pbias"], "Dbias": tabs["Dbias"],
        "W2sT": tabs["W2sT"], "ob": tabs["ob"],
    }
    in_maps = [dict(shared, xim=xims[c]) for c in range(N_CORES)]
    res = run_bass_kernel_spmd(nc, in_maps, core_ids=list(range(N_CORES)),
                               **_CACHE.get("run_kwargs", {}))
    _CACHE["last_result"] = res
    out = np.concatenate([res.results[c]["out"] for c in range(N_CORES)], axis=0)
    return out.astype(np.float32)


# revision 14
# speedup vs baseline: 1.5499x; 1.5499x over previous
"""CapsNet forward as a fused Bass/Tile kernel on 8 Trainium2 NeuronCores.

Math (same collapse as before, validated vs jax reference):
  routing logits never update -> uniform c -> capsule stage = mean over the
  1152 capsules; squash scalars; tiny logits matmul; softmax.

v2 structure (per core, 64 samples, 4 chunks of 16):
  - im2col is done HOST-side: xim[k] = [81, 19, 20, 16] bf16 (x col 19 = 0),
    one contiguous SWDGE DMA per chunk (no gather, no memsets).
  - conv1: per y-row matmul K=81 N=320 (bf16), evicted with relu+bias to
    C1t[T] in PHASE-MAJOR fp8e4 layout [128=(dd,ci), phi, y', xm, xq, b]
    (y = 4*y' + phi, x = 4*xq + xm).  Phase-major makes each (T,dd) shuffle
    source a single contiguous 6400B run per partition.
  - shuffle: 8 DMAs/chunk (one per T,dd): C1t[T][32dd:32dd+32, :] ->
    C1ph[:, d] with dst partitions ordered (ci,phi) = 4*ci+phi; contiguous
    1600B runs both sides.
  - prim caps conv: fp8e4 DoubleRow; the 33 K-tiles pair into 16 DR pairs
    (11 seg0+seg1 pairs, 5 seg2 xh-pairs) + 1 single.  Weights prescaled
    x64 host-side; oy-inner loop reuses each loaded weight pair for 3 MMs.
    Evict relu(psum/64 + b) to U bf16.
  - dig projection / squash / logits / softmax: as before.
"""
import sys

sys.path.insert(0, "/opt/trn_rl_repo")

import numpy as np
import ml_dtypes

N_CORES = 8
B = 512
BC = B // N_CORES        # 64 samples per core
BCH = 16                 # batch chunk
NCH = BC // BCH          # 4 chunks

W2SCALE = 64.0
SWI = False   # use DoubleRowSwInterleave weight layout for the prim conv

# DR pair table: 16 pairs + 1 single over the 33 prim K-tiles (xh, seg).
# pairs i<11: ((i,0),(i,1)) stride = y'+1; pairs 11..15: ((2j,2),(2j+1,2))
# stride = xm+1; single: (10,2).
N_PAIRS = 16


def _pair_members(i):
    if i < 11:
        return (i, 0), (i, 1)
    j = i - 11
    return (2 * j, 2), (2 * j + 1, 2)


# ---------------------------------------------------------------- host tables
def _build_tables(conv1_w, conv1_b, prim_w, prim_b, dig_W, dig_Wb, out_w, out_b):
    bf = ml_dtypes.bfloat16
    f8 = ml_dtypes.float8_e4m3
    w1 = conv1_w[:, 0].reshape(256, 81)
    w2 = prim_w[:, :, 0]                       # [co, ci, p, q]

    W1r = np.zeros((81, 256), np.float32)
    Cbias = np.zeros((128, 2), np.float32)
    for d in range(8):
        for ci in range(32):
            c = ci * 8 + d
            T, mu = d // 4, 32 * (d % 4) + ci
            W1r[:, T * 128 + mu] = w1[c]
            Cbias[mu, T] = conv1_b[c]

    # W2stk[row=(4*ci+phi), pair, member, col=(32*s+co)] fp8, x64
    W2stk = np.zeros((128, 17, 2, 128), np.float32)

    def fill_tile(pr, m, xh, seg):
        nphi = 4 if seg < 2 else 3
        for phi in range(nphi):
            for s in range(4):
                sy, sx = s // 2, s % 2
                p = 4 * seg + phi - 2 * sy
                q = xh - 2 * sx
                if 0 <= p <= 8 and 0 <= q <= 8:
                    for ci in range(32):
                        W2stk[4 * ci + phi, pr, m, 32 * s:32 * s + 32] = \
                            w2[:, ci, p, q] * W2SCALE

    for i in range(N_PAIRS):
        (xa, sa), (xb, sb) = _pair_members(i)
        fill_tile(i, 0, xa, sa)
        fill_tile(i, 1, xb, sb)
    fill_tile(16, 0, 10, 2)

    Pbias = np.zeros((128, 1), np.float32)
    for s in range(4):
        Pbias[32 * s:32 * s + 32, 0] = prim_b

    Wdig = np.zeros((72, 128, 16), np.float32)
    t = 0
    for oy in range(3):
        for d in range(8):
            for ox in range(3):
                for s in range(4):
                    sy, sx = s // 2, s % 2
                    ip, jp = 2 * oy + sy, 2 * ox + sx
                    for co in range(32):
                        n = co * 36 + jp * 6 + ip
                        Wdig[t, 32 * s + co] = dig_W[n, d] / 1152.0
                t += 1

    return dict(
        W1r=W1r.astype(bf),
        Cbias=Cbias,
        W2stk=W2stk.astype(f8),
        Pbias=Pbias,
        Wdig=np.ascontiguousarray(Wdig.transpose(1, 0, 2)).astype(bf),  # [128,72,16]
        Dbias=(dig_Wb.sum(0) / 1152.0).reshape(16, 1).astype(np.float32),
        W2sT=np.ascontiguousarray(out_w[..., 0].sum(1).T).astype(np.float32),
        ob=np.tile(out_b[None, :], (BC, 1)).astype(np.float32),
    )


def _build_xim(x):
    """Host im2col: per core, [NCH, 81, 19, 20, BCH] bf16, x col 19 zeroed."""
    from numpy.lib.stride_tricks import sliding_window_view
    xi = x[:, 0]                                        # [B, 28, 28] f32
    out = []
    for c in range(N_CORES):
        xc = xi[c * BC:(c + 1) * BC]                    # [64, 28, 28]
        w = sliding_window_view(xc, (9, 9), axis=(1, 2))[:, :19, :19]  # [64,19,19,9,9]
        im = np.zeros((81, 19, 20, NCH, BCH), np.float32)
        im[:, :, :19] = (
            w.transpose(3, 4, 1, 2, 0).reshape(81, 19, 19, NCH, BCH)
        )
        out.append(np.ascontiguousarray(
            im.transpose(3, 0, 1, 2, 4)).astype(ml_dtypes.bfloat16))
    return out                                           # list of [4,81,19,20,16]


# ---------------------------------------------------------------- bass kernel
def _build_nc():
    import concourse.bacc as bacc
    import concourse.bass as bass
    import concourse.mybir as mybir
    import concourse.tile as tile
    from concourse.masks import make_identity

    bf = mybir.dt.bfloat16
    f8 = mybir.dt.float8e4
    f32 = mybir.dt.float32
    AF = mybir.ActivationFunctionType
    AX = mybir.AxisListType
    DR = mybir.MatmulPerfMode.DoubleRow

    nc = bacc.Bacc(None, target_bir_lowering=False)

    xim_in = nc.dram_tensor("xim", [NCH, 81, 19, 20, BCH], bf, kind="ExternalInput")
    W1r_d = nc.dram_tensor("W1r", [81, 256], bf, kind="ExternalInput")
    W2stk_d = nc.dram_tensor("W2stk", [128, 17, 2, 128], f8, kind="ExternalInput")
    Wdig_d = nc.dram_tensor("Wdig", [128, 72, 16], bf, kind="ExternalInput")
    Cbias_d = nc.dram_tensor("Cbias", [128, 2], f32, kind="ExternalInput")
    Pbias_d = nc.dram_tensor("Pbias", [128, 1], f32, kind="ExternalInput")
    Dbias_d = nc.dram_tensor("Dbias", [16, 1], f32, kind="ExternalInput")
    W2sT_d = nc.dram_tensor("W2sT", [16, 10], f32, kind="ExternalInput")
    ob_d = nc.dram_tensor("ob", [BC, 10], f32, kind="ExternalInput")
    out_d = nc.dram_tensor("out", [BC, 10], f32, kind="ExternalOutput")

    with tile.TileContext(nc) as tc:
        with (
            tc.tile_pool(name="consts", bufs=1) as consts,
            tc.tile_pool(name="work", bufs=2) as work,
            tc.tile_pool(name="usb", bufs=1) as usbp,
            tc.tile_pool(name="fin", bufs=1) as fin,
            tc.tile_pool(name="ps1", bufs=3, space="PSUM") as ps1p,
            tc.tile_pool(name="pspr", bufs=3, space="PSUM") as psprp,
            tc.tile_pool(name="psm", bufs=1, space="PSUM") as psmp,
            tc.tile_pool(name="pssm", bufs=1, space="PSUM") as pssmp,
            nc.allow_non_contiguous_dma("phase shuffle partition regroup"),
        ):
            # ---- constants: small ones first so conv1 can start early;
            # the big prim/dig tables load behind them on the sync ring.
            W1sb = consts.tile([81, 256], bf)
            nc.sync.dma_start(out=W1sb, in_=W1r_d[:, :])
            Cb = consts.tile([128, 2], f32)
            nc.scalar.dma_start(out=Cb, in_=Cbias_d[:, :])
            Pb = consts.tile([128, 1], f32)
            nc.scalar.dma_start(out=Pb, in_=Pbias_d[:, :])
            Db = consts.tile([16, 1], f32)
            nc.scalar.dma_start(out=Db, in_=Dbias_d[:, :])
            W2s = consts.tile([16, 10], f32)
            nc.scalar.dma_start(out=W2s, in_=W2sT_d[:, :])
            obt = consts.tile([BC, 10], f32)
            nc.scalar.dma_start(out=obt, in_=ob_d[:, :])
            W2sb = consts.tile([128, 17, 2, 128], f8)
            nc.sync.dma_start(out=W2sb, in_=W2stk_d[:, :, :, :])
            Wdsb = consts.tile([128, 72, 16], bf)
            nc.sync.dma_start(out=Wdsb, in_=Wdig_d[:, :, :])
            idf = consts.tile([16, 16], f32)
            make_identity(nc, idf)

            U_sb = usbp.tile([128, 3, 8, 3, BC], bf)

            # ---------------- prim caps helpers (fp8 DoubleRow) ----------
            def pair_rhs(C1ph, i, oy):
                if i < 11:
                    xm0, xq0 = i % 4, i // 4
                    s = C1ph[:, :, oy:oy + 2, xm0, xq0:xq0 + 3, :]
                else:
                    xh = 2 * (i - 11)
                    xm0, xq0 = xh % 4, xh // 4
                    s = C1ph[:, :, oy + 2, xm0:xm0 + 2, xq0:xq0 + 3, :]
                return s.transpose([0, 2, 1, 3, 4])

            def emit_prim_mm(C1ph, i, oy, pspr):
                if i < N_PAIRS:
                    nc.tensor.matmul(
                        pspr[:, :, :, :],
                        W2sb[:, i, :, :], pair_rhs(C1ph, i, oy),
                        start=(i == 0), stop=False, perf_mode=DR)
                else:
                    nc.tensor.matmul(
                        pspr[:, :, :, :],
                        W2sb[:, 16, 0, :],
                        C1ph[:, :, oy + 2, 2, 2:5, :],
                        start=False, stop=True)

            def evict_U(k, oy, pspr):
                nc.scalar.activation(
                    out=U_sb[:, oy, :, :, k * BCH:(k + 1) * BCH],
                    in_=pspr, func=AF.Relu, bias=Pb[:, 0:1], scale=1.0 / W2SCALE)

            def prim_items(C1ph, psprs, k):
                # oy-outer: the first 17 MMs (oy=0) read only y' 0:2 of C1ph,
                # i.e. depend on the low-half shuffles only -> a ~17-MM runway
                # that hides the high-half shuffle latency at chunk start.
                for oy in range(3):
                    for i in range(17):
                        yield ('mm', C1ph, i, oy, psprs[oy])
                    yield ('ev', k, oy, psprs[oy])

            def run_item(it):
                if it[0] == 'mm':
                    _, C1ph, i, oy, pspr = it
                    emit_prim_mm(C1ph, i, oy, pspr)
                else:
                    _, k, oy, pspr = it
                    evict_U(k, oy, pspr)

            # ---------------- conv1 + phase-major evict ------------------
            def emit_conv1_T(im1, T, C1t_T, C1ph, inter):
                for y in range(19):
                    ps = ps1p.tile([128, 20, BCH], f32, tag="ps1", name="ps1")
                    nc.tensor.matmul(ps, W1sb[:, 128 * T:128 * (T + 1)],
                                     im1[:, y, :, :])
                    phi, yp = y % 4, y // 4
                    dst = C1t_T[:, phi, yp, :, :, :]
                    src = ps.rearrange("p (xq xm) b -> p xm xq b", xm=4)
                    if y % 2 == 0:
                        nc.scalar.activation(out=dst, in_=src, func=AF.Relu,
                                             bias=Cb[:, T:T + 1], scale=1.0)
                    else:
                        nc.vector.tensor_scalar(
                            out=dst, in0=src, scalar1=Cb[:, T:T + 1],
                            scalar2=0.0, op0=mybir.AluOpType.add,
                            op1=mybir.AluOpType.max)
                    if y == 11:
                        # rows y<=11 = y' 0:3 of every phi are evicted: the
                        # low-half shuffle can start while conv1 finishes
                        emit_shuffle_T(T, C1t_T, C1ph, 0, 3)
                    inter(y)

            def emit_shuffle_T(T, C1t_T, C1ph, lo, hi):
                # y'-slice [lo:hi] of the phase shuffle: the low slice only
                # needs conv1 rows y <= 4*(lo... issued mid-conv1 so most of
                # the shuffle overlaps compute.
                for dd in range(4):
                    d = 4 * T + dd
                    eng = nc.sync if dd % 2 == 0 else nc.scalar
                    eng.dma_start(
                        out=C1ph[:, d, lo:hi, :, :, :],
                        in_=C1t_T[32 * dd:32 * dd + 32, :, lo:hi, :, :, :])

            def emit_im1(k):
                # SWDGE (gpsimd) spreads these tall [81 x 12KB] transfers
                # over all 16 SDMA engines; the HWDGE rings serialize them
                # onto one engine (measured 137 vs 20 GB/s).
                im1 = work.tile([81, 19, 20, BCH], bf, tag="im1", name="im1",
                                bufs=2)
                if k == 0:
                    # split so conv1 y<10 can start before the whole chunk lands
                    nc.gpsimd.dma_start(out=im1[:, 0:10, :, :],
                                        in_=xim_in[k, :, 0:10, :, :])
                    nc.gpsimd.dma_start(out=im1[:, 10:19, :, :],
                                        in_=xim_in[k, :, 10:19, :, :])
                else:
                    nc.gpsimd.dma_start(out=im1[:, :, :, :],
                                        in_=xim_in[k, :, :, :, :])
                return im1

            # ---------------- chunk loop --------------------------------
            im1s = {0: emit_im1(0)}
            pend = None      # prim work-item iterator for chunk k-1
            for k in range(NCH):
                im1 = im1s.pop(k)
                C1t = [work.tile([128, 4, 5, 4, 5, BCH], f8, tag=f"c1t{T}",
                                 name=f"c1t{T}") for T in range(2)]
                if k < 2:
                    # zero the phi3/y'4 pad slot once per pool buffer so the
                    # shuffle copies finite data into C1ph's unused slots
                    # (read with zero weights by the seg2 K-tiles).
                    for T in range(2):
                        nc.gpsimd.memset(C1t[T][:, 3, 4, :, :, :], 0.0)
                C1ph = work.tile([128, 8, 5, 4, 5, BCH], f8, tag="c1ph",
                                 name="c1ph", bufs=3)

                def inter(y):
                    if pend is not None and y % 2 == 1:
                        for _ in range(3):
                            it = next(pend, None)
                            if it is not None:
                                run_item(it)

                emit_conv1_T(im1, 0, C1t[0], C1ph, inter)
                emit_shuffle_T(0, C1t[0], C1ph, 3, 5)
                if k + 1 < NCH:
                    # prefetch after chunk 0's load has had the queue to itself
                    im1s[k + 1] = emit_im1(k + 1)
                emit_conv1_T(im1, 1, C1t[1], C1ph, inter)
                emit_shuffle_T(1, C1t[1], C1ph, 3, 5)
                if pend is not None:
                    for it in pend:
                        run_item(it)

                psprs = [psprp.tile([128, 8, 3, BCH], f32, tag="pspr",
                                    name="pspr") for _ in range(3)]
                pend = prim_items(C1ph, psprs, k)

            # ---------------- tail: chunk 3 prim (oy-outer) + dig --------
            psm = psmp.tile([16, BC], f32, tag="psm")
            tdig = [0]

            def dig_gen(oy):
                # yields after emitting each dig matmul so it can interleave
                # with the next oy's prim matmuls
                for d in range(8):
                    for ox in range(3):
                        t = tdig[0]
                        nc.tensor.matmul(psm, Wdsb[:, t, :],
                                         U_sb[:, oy, d, ox, :],
                                         start=(t == 0), stop=(t == 71))
                        tdig[0] += 1
                        yield

            dig_it = None
            for it in pend:
                run_item(it)
                if it[0] == 'mm':
                    if dig_it is not None:
                        next(dig_it, None)
                else:
                    if dig_it is not None:
                        for _ in dig_it:
                            pass
                    dig_it = dig_gen(it[2])
            for _ in dig_it:
                pass

            # ---------------- squash / logits / softmax ------------------
            m_sb = fin.tile([16, BC], f32)
            nc.vector.tensor_scalar_add(out=m_sb, in0=psm, scalar1=Db[:, 0:1])

            psT = pssmp.tile([BC, 16], f32, tag="pssm")
            nc.tensor.transpose(psT, m_sb, idf)
            mT = fin.tile([BC, 16], f32)
            nc.vector.tensor_copy(out=mT, in_=psT)
            sq = fin.tile([BC, 16], f32)
            nc.vector.tensor_mul(sq, mT, mT)
            l2 = fin.tile([BC, 1], f32)
            nc.vector.reduce_sum(l2, sq, axis=AX.X)
            nc.scalar.activation(out=l2, in_=l2, func=AF.Sqrt)
            l1 = fin.tile([BC, 1], f32)
            nc.vector.tensor_reduce(l1, mT, axis=AX.X, op=mybir.AluOpType.add,
                                    apply_absolute_value=True)
            den = fin.tile([BC, 1], f32)
            nc.vector.tensor_scalar_add(out=den, in0=l2, scalar1=1.0)
            nc.vector.tensor_mul(den, den, l1)
            rden = fin.tile([BC, 1], f32)
            nc.vector.reciprocal(rden, den)
            scl = fin.tile([BC, 1], f32)
            nc.vector.tensor_mul(scl, l2, rden)

            pslg = pssmp.tile([BC, 10], f32, tag="pssm")
            nc.tensor.matmul(pslg, m_sb, W2s)
            lg = fin.tile([BC, 10], f32)
            nc.vector.tensor_scalar_mul(out=lg, in0=pslg, scalar1=scl[:, 0:1])
            nc.vector.tensor_add(lg, lg, obt)
            mx = fin.tile([BC, 1], f32)
            nc.vector.reduce_max(mx, lg, axis=AX.X)
            nc.vector.tensor_scalar_sub(out=lg, in0=lg, scalar1=mx[:, 0:1])
            ex = fin.tile([BC, 10], f32)
            nc.scalar.activation(out=ex, in_=lg, func=AF.Exp)
            sm = fin.tile([BC, 1], f32)
            nc.vector.reduce_sum(sm, ex, axis=AX.X)
            rsm = fin.tile([BC, 1], f32)
            nc.vector.reciprocal(rsm, sm)
            outt = fin.tile([BC, 10], f32)
            nc.vector.tensor_scalar_mul(out=outt, in0=ex, scalar1=rsm[:, 0:1])
            nc.sync.dma_start(out=out_d[:, :], in_=outt)

    nc.finalize()
    return nc


_CACHE = {}


def kernel(**inputs):
    from concourse.bass_utils import run_bass_kernel_spmd

    np_in = {k: np.asarray(v) for k, v in inputs.items()}
    tabs = _build_tables(
        np_in["conv1_w"], np_in["conv1_b"], np_in["prim_w"], np_in["prim_b"],
        np_in["dig_W"], np_in["dig_Wb"], np_in["out_w"], np_in["out_b"],
    )
    xims = _build_xim(np_in["x"].astype(np.float32))

    if "nc" not in _CACHE:
        _CACHE["nc"] = _build_nc()
    nc = _CACHE["nc"]

    shared = {
        "W1r": tabs["W1r"], "W2stk": tabs["W2stk"], "Wdig": tabs["Wdig"],
        "Cbias": tabs["Cbias"], "Pbias": tabs["Pbias"], "Dbias": tabs["Dbias"],
        "W2sT": tabs["W2sT"], "ob": tabs["ob"],
    }
    in_maps = [dict(shared, xim=xims[c]) for c in range(N_CORES)]
    res = run_bass_kernel_spmd(nc, in_maps, core_ids=list(range(N_CORES)),
                               **_CACHE.get("run_kwargs", {}))
    _CACHE["last_result"] = res
    out = np.concatenate([res.results[c]["out"] for c in range(N_CORES)], axis=0)
    return out.astype(np.float32)
